# revision 106
# baseline (speedup 1.0000x reference)
"""Trainium2 Bass kernel for a causal single-head attention block.

Reference computation (per batch b):
    q = x @ Wq + bq ; k = x @ Wk + bk ; v = x @ Wv + bv      (x: [S, D])
    logits = q @ k.T  (causal masked), probs = softmax(logits / sqrt(128))
    out = concat([x, probs @ v], axis=-1)                     -> [S, D+128]

Shapes are hardcoded: B=4, S=2048, D=1024, feature size 128, 8 NeuronCores.

Sharding (SPMD, one compiled graph for all 8 cores):
  core c -> batch b = c//2, interleave parity h = c%2.
  Each core computes the 8 query blocks (128 rows each) at global block
  positions {2j + h : j in 0..7} of its batch, plus the K/V projection over
  the full 2048-row sequence of that batch.  The passthrough half of the
  output (out[:, :D] = x) is assembled on the host; the device returns only
  the attention read [1024, 128] per core.

  To keep the causal block structure identical across cores (SPMD requires
  one instruction stream), the host feeds h=1 cores a pair-swapped column
  order of x^T.  In local block coordinates every core then has: query
  blocks at even local positions 2j, valid key blocks lk < 2j+2, a
  triangular causal mask on key slot lk=2j, and a slot lk=2j+1 that is
  fully masked for h=0 / fully valid for h=1.  The two 128x128 mask tiles
  are per-core input data.

On-chip scheme:
  - x^T shipped as fp8e4m3 (2MB/core) in a host-permuted column order:
    storage blocks 0..7 = the core's query blocks (even local), 8..15 = the
    odd blocks; chunk-major flat so every DMA is one contiguous block.
    The query half streams in first, so the whole exp pipeline (the
    bottleneck engine) starts ~4us into the kernel and never waits on the
    causally-late keys.
  - Projection weights fp8 scaled x32 (avoids fp8 subnormals; W ~N(0,1/D)).
    K/Q projections are DoubleRow fp8 matmuls (2 contraction rows per
    partition) producing 32*k^T / 32*q^T in bf16; v is produced directly in
    NATURAL layout (x8 block as the stationary operand, Wv moving) -- no PE
    transposes.  The x32 scales cancel: the exp scale absorbs 1/1024, the
    PV denominator column is 32.0, and bv is added on the host.
  - logits computed transposed per (query block j, <=4 key slots): one PSUM
    bank per group, one exp each (24 exps total -- the activation engine is
    the bottleneck, so exp instruction count is minimized).  Causal masks
    are multiplicative 0/1 AFTER the exp (exp(x+m) == exp(x)*[m==0]) and
    run on gpsimd (tail-critical ones on DVE); the garbage logits at
    (slot > j) are never read by PV.
  - PV + softmax denominator accumulate per query block j over its 2j+2
    valid slots into one bank ([128,129], ones-column trick); reciprocal
    normalize on DVE; query blocks 2m,2m+1 finish and stream out as odd
    chunk 4+m lands.
  - Engine budget: Act = exps only (plus 2 early Q evacs that fill its
    startup gaps); DVE = K/Q evacs, vnat evacs, normalize; gpsimd = masks;
    all input DMAs on the sync queue (a DMA issued from the scalar queue
    would occupy the activation engine).
"""

import math

import numpy as np

import concourse.tile as tile
from concourse import bacc, mybir
from concourse.bass_utils import run_bass_kernel_spmd
N_CORES = 8
B = 4
S = 2048  # sequence length per batch
D = 1024  # model dim
F = 128  # q/k/v feature size
NQT = 8  # local query subtiles of 128 rows
NKT = 16  # key tiles of 128 rows (full sequence)
QROWS = NQT * 128  # 1024 local query rows per core
WSCALE = 32.0  # fp8 weight prescale
SCALE = 1.0 / math.sqrt(F) / (WSCALE * WSCALE)  # exp scale on the logits psum

FP32 = mybir.dt.float32
BF16 = mybir.dt.bfloat16
FP8 = mybir.dt.float8e4
BF16_NP = mybir.dt.np(BF16)
FP8_NP = mybir.dt.np(FP8)

# storage-column chunks; x8 ships chunk-major flat so every DMA is one
# contiguous block.  Tiny head chunks start the exp pipeline earliest;
# fine granularity keeps it fed.
CHUNKS = tuple((256 * c, 256) for c in range(8))

_compiled = {}


def _build():
    nc = bacc.Bacc("TRN2", target_bir_lowering=False, debug=False, num_devices=N_CORES)

    x8_ext = nc.dram_tensor("x8", [128, 8 * S], FP8, kind="ExternalInput")
    # packed weights: [k|q|v][d-tile][feature], fp8 x32
    w_ext = nc.dram_tensor("w3", [128, 3, 8, 128], FP8, kind="ExternalInput")
    # packed biases x32: [k|q|v]
    b_ext = nc.dram_tensor("b3", [128, 3], FP32, kind="ExternalInput")
    # multiplicative masks (0/1): slot 0 diag triangle, slot 1 parity block
    mask_ext = nc.dram_tensor("masks", [128, 2, 128], BF16, kind="ExternalInput")
    # partition-major output: out[p, j, f] = read row j*128+p
    out_ext = nc.dram_tensor("out", [128, NQT, F], BF16, kind="ExternalOutput")

    DR = mybir.MatmulPerfMode.DoubleRow

    with tile.TileContext(nc) as tc:
        # x8 column storage order (host-permuted): storage block s in 0..7
        # holds local even block 2s (the query blocks); s in 8..15 holds
        # local odd block 2(s-8)+1.  All of qT exists after storage chunk 1,
        # so the exp workload front-loads instead of bunching behind the
        # last chunk.
        with (
            tc.tile_pool(name="persist", bufs=1) as P,
            tc.tile_pool(name="ps_proj", bufs=3, space="PSUM") as ps_proj,
            tc.tile_pool(name="ps_log", bufs=3, space="PSUM") as ps_log,
            tc.tile_pool(name="ps_pv", bufs=2, space="PSUM") as ps_pv,
        ):
            # ---- persistent SBUF tiles ----
            x8_sb = P.tile([128, 8 * S], FP8)  # chunk-major [c][d//128, s]
            w_sb = P.tile([128, 3, 8, 128], FP8)  # [d%128, kqv, d//128, f]
            b_sb = P.tile([128, 3], FP32)
            mask_sb = P.tile([128, 2, 128], BF16)
            zero_sb = P.tile([128, 1], FP32)
            kT_sb = P.tile([128, S], BF16)  # [feat, s] = 32*k^T
            qT_sb = P.tile([128, QROWS], BF16)  # [feat, local q] = 32*q^T
            vaug_sb = P.tile([128, NKT, 132], BF16)  # [s%128, ki, 32*vfeat|32]
            expT_sb = P.tile([128, NKT, QROWS], BF16)  # [s%128, ki, local q]
            read_sb = P.tile([128, NQT, 128], BF16)
            recip_sb = P.tile([128, NQT, 1], FP32)

            # constants computed on-chip first (DVE is idle at t=0)
            nc.vector.memset(zero_sb[:], 0.0)
            nc.vector.memset(vaug_sb[:, :, 128:129], WSCALE)



            # ---- input DMAs, in consumption order (the DMA device is
            # effectively serial): wk, biases, chunk 0, wq, chunks 1-7.
            # All on the sync queue -- a DMA issued from the scalar queue
            # would occupy the activation engine, which must stay free for
            # the exp pipeline.  wv and the masks ride the otherwise-idle
            # SWDGE queue; their consumers run far behind the exp path. ----
            wflat_sb = w_sb[:].rearrange("p a b c -> p (a b c)")
            wflat_ext = w_ext[:].rearrange("p a b c -> p (a b c)")
            nc.sync.dma_start(wflat_sb[:, 0:1024], wflat_ext[:, 0:1024])

            def chunk_dma(eng, c):
                off, w = CHUNKS[c]
                base = 8 * off
                eng.dma_start(
                    x8_sb[:, base:base + 8 * w], x8_ext[:, base:base + 8 * w]
                )

            nc.sync.dma_start(b_sb[:], b_ext[:])
            chunk_dma(nc.sync, 0)
            nc.sync.dma_start(wflat_sb[:, 1024:2048], wflat_ext[:, 1024:2048])
            nc.gpsimd.dma_start(wflat_sb[:, 2048:3072], wflat_ext[:, 2048:3072])
            for c in range(1, len(CHUNKS)):
                chunk_dma(nc.sync, c)
            nc.gpsimd.dma_start(
                mask_sb[:].rearrange("p a b -> p (a b)"),
                mask_ext[:].rearrange("p a b -> p (a b)"),
            )

            def xv(c):
                # chunk c of x8 as [128, t, w]
                off, w = CHUNKS[c]
                base = 8 * off
                return x8_sb[:, base:base + 8 * w].rearrange(
                    "p (t w) -> p t w", t=8
                )

            # ---- projections, chunk-paced.  All matmuls fp8 DoubleRow:
            # stationary [128, 2, 128] (pair of d-tiles), moving
            # [128, 2, w].  K/V evacuations (bias add, bf16 out) run on the
            # scalar engine (idle until the exp phase); Q evacs on DVE. ----
            def proj(kqv, dst_ap, c, evac, split=False):
                # project chunk c (its full width) through W[kqv]
                width = CHUNKS[c][1]
                xc = xv(c)
                pp = ps_proj.tile([128, 512], FP32, tag="proj", name="pp")
                for t2 in range(4):
                    nc.tensor.matmul(
                        pp[:, 0:width],
                        w_sb[:, kqv, 2 * t2:2 * t2 + 2, :],
                        xc[:, 2 * t2:2 * t2 + 2, :],
                        start=(t2 == 0),
                        stop=(t2 == 3),
                        perf_mode=DR,
                    )
                if split:
                    # half-wise: the first 128 cols unlock logits(j0) early
                    evac(dst_ap[:, 0:128], pp[:, 0:128], b_sb[:, kqv:kqv + 1])
                    evac(dst_ap[:, 128:width], pp[:, 128:width],
                         b_sb[:, kqv:kqv + 1])
                else:
                    evac(dst_ap, pp[:, 0:width], b_sb[:, kqv:kqv + 1])

            def vnat(s):
                # v in natural layout, directly: out[s_row, vf] accumulates
                # x8 d-pairs as the stationary operand, Wv pairs moving.
                # (v is scaled x32; bv is added on the host.)
                c = next(i for i, (o, w) in enumerate(CHUNKS) if o <= s * 128 < o + w)
                loc = s * 128 - CHUNKS[c][0]
                xc = xv(c)
                pv = ps_proj.tile([128, 512], FP32, tag="proj", name="pv")
                for t2 in range(4):
                    nc.tensor.matmul(
                        pv[:, 0:128],
                        xc[:, 2 * t2:2 * t2 + 2, loc:loc + 128],
                        w_sb[:, 2, 2 * t2:2 * t2 + 2, :],
                        start=(t2 == 0),
                        stop=(t2 == 3),
                        perf_mode=DR,
                    )
                nc.vector.tensor_copy(vaug_sb[:, s, 0:128], pv[:, 0:128])

            out_read = out_ext[:]

            def logit_exp(j, s0, slen):
                # logits^T for query block j against storage key slots
                # s0..s0+slen, one PSUM bank, one exp
                jsl = slice(j * 128, (j + 1) * 128)
                pl = ps_log.tile([128, 512], FP32, tag="log", name="pl")
                pl4 = pl.rearrange("p (a b) -> p a b", b=128)
                for gi in range(slen):
                    nc.tensor.matmul(
                        pl4[:, gi, :],
                        kT_sb[:, (s0 + gi) * 128:(s0 + gi + 1) * 128],
                        qT_sb[:, jsl],
                        start=True, stop=True,
                    )
                nc.scalar.activation(
                    expT_sb[:, s0:s0 + slen, jsl],
                    pl4[:, 0:slen, :],
                    mybir.ActivationFunctionType.Exp, bias=zero_sb[:],
                    scale=SCALE,
                )

            def mask_mul(j, side, eng=None):
                # multiplicative causal mask on the exp'd slot:
                # exp(x+m) == exp(x)*[m==0].  side 0 = diag triangle on
                # even slot j, side 1 = parity block on odd slot 8+j.
                # SBUF-to-SBUF, so the idle gpsimd engine carries most;
                # tail-critical ones run on the faster DVE.
                jsl = slice(j * 128, (j + 1) * 128)
                s = j + 8 * side
                (eng or nc.gpsimd).tensor_mul(
                    expT_sb[:, s, jsl],
                    expT_sb[:, s, jsl],
                    mask_sb[:, side, :],
                )

            def finish(j, pool=None, tag="pv"):
                # PV chain over the 2j+2 valid storage slots (masked slot
                # 8+j last), normalize
                jsl = slice(j * 128, (j + 1) * 128)
                pr = (pool or ps_pv).tile([128, 512], FP32, tag=tag, name="pr")
                slots = list(range(j + 1)) + list(range(8, 9 + j))
                for n, s in enumerate(slots):
                    nc.tensor.matmul(
                        pr[:, 0:129],
                        expT_sb[:, s, jsl],
                        vaug_sb[:, s, 0:129],
                        start=(n == 0),
                        stop=(n == len(slots) - 1),
                    )
                nc.vector.reciprocal(recip_sb[:, j, :], pr[:, 128:129])
                nc.vector.tensor_scalar_mul(
                    read_sb[:, j, :], pr[:, 0:128], recip_sb[:, j, :]
                )

            def act_evac(dst, src, bias):
                # Q evac on the scalar engine: feeds that engine's own
                # exp pipeline, in parallel with K's evac on DVE
                nc.scalar.activation(
                    dst, src, mybir.ActivationFunctionType.Identity, bias=bias
                )

            # chunks 0-4 carry the query blocks (j0 | j1 | j2,j3 | j4,j5 |
            # j6,j7 plus their k/v columns); chunks 5-8 the odd (non-query)
            # key blocks (2 odd slots each).  Each chunk's exp-path work
            # (K evac on DVE, Q evac on Act early / DVE later, logits,
            # exp) is emitted before its v-path work; query blocks
            # 2m/2m+1 finish as odd chunk 5+m lands.  Odd-side exps use
            # the widest groups available (Act runs a backlog, so later
            # readiness of a wider group costs nothing but saves
            # instruction overhead on the bottleneck engine).
            for c, (off, w) in enumerate(CHUNKS):
                sl = slice(off, off + w)
                proj(0, kT_sb[:, sl], c, nc.vector.tensor_scalar_add)
                if off < QROWS:
                    proj(1, qT_sb[:, sl], c,
                         act_evac if c <= 1 else nc.vector.tensor_scalar_add)

                if c < 4:
                    for j in (2 * c, 2 * c + 1):
                        logit_exp(j, 0, min(j + 1, 4))
                        if j > 3:
                            logit_exp(j, 4, j - 3)
                        mask_mul(j, 0)
                    if c == 3:
                        for s in range(8):
                            vnat(s)
                else:
                    m = c - 4
                    if c == 4:
                        for j in (1, 0):
                            logit_exp(j, 8, j + 1)
                    elif c == 5:
                        for j in range(NQT - 1, 1, -1):
                            logit_exp(j, 8, min(j + 1, 4))
                    elif c == 6:
                        for j in (5, 4):
                            logit_exp(j, 12, j - 3)
                    else:
                        for j in (7, 6):
                            logit_exp(j, 12, j - 3)
                    s0 = 8 + 2 * m
                    meng = nc.vector if m >= 2 else None
                    mask_mul(2 * m + 1, 1, meng)
                    mask_mul(2 * m, 1, meng)
                    vnat(s0)
                    vnat(s0 + 1)
                    finish(2 * m + 1)
                    finish(2 * m, ps_proj, "proj")
                    nc.sync.dma_start(
                        out=out_read[:, 2 * m:2 * m + 2, :],
                        in_=read_sb[:, 2 * m:2 * m + 2, :],
                    )

    nc.compile()
    return nc


def _get_compiled():
    if "nc" not in _compiled:
        _compiled["nc"] = _build()
    return _compiled["nc"]


def _make_in_maps(inputs, Wq, bq, Wk, bk, Wv, bv):
    x = np.asarray(inputs, dtype=np.float32)
    assert x.shape == (B, S, D)

    def prep_w(w):
        w = (np.asarray(w, dtype=np.float32) * WSCALE).astype(FP8_NP)
        return w.reshape(8, 128, 128).transpose(1, 0, 2)  # [d%128, t, f]

    w3_np = np.ascontiguousarray(
        np.stack([prep_w(Wk), prep_w(Wq), prep_w(Wv)], axis=1)
    )  # [128, kqv, 8, 128]
    b3_np = np.ascontiguousarray(
        np.stack(
            [np.asarray(b, np.float32).reshape(128) * WSCALE for b in (bk, bq, bv)],
            axis=1,
        )
    )  # [128, kqv]

    # multiplicative masks[k, slot, q] (0 = masked): slot 0 = diagonal block
    # (triangle), slot 1 = parity block (all-0 for h=0, all-1 for h=1)
    kk = np.arange(128)[:, None]
    qq = np.arange(128)[None, :]
    tri = (qq >= kk).astype(np.float32)
    m_h = []
    for h in range(2):
        other = np.full((128, 128), 0.0 if h == 0 else 1.0, np.float32)
        m = np.stack([tri, other], axis=1)  # [k, slot, q]
        m_h.append(np.ascontiguousarray(m.astype(BF16_NP)))

    # storage order: even local blocks (queries) first, odd blocks second
    storage = np.concatenate([np.arange(0, NKT, 2), np.arange(1, NKT, 2)])
    in_maps = []
    for c in range(N_CORES):
        b, h = divmod(c, 2)
        xb = x[b]  # [S, D]
        # local block order: pair-swap for h=1
        order = np.arange(NKT) if h == 0 else (np.arange(NKT) ^ 1)
        xb_local = xb.reshape(NKT, 128, D)[order][storage].reshape(S, D)
        xT_full = xb_local.T.astype(FP8_NP)  # [D, S]
        xT_tps = xT_full.reshape(8, 128, S).transpose(1, 0, 2)  # [p, t, s]
        x8 = np.concatenate(
            [xT_tps[:, :, off:off + w].reshape(128, -1) for off, w in CHUNKS],
            axis=1,
        )  # chunk-major flat [p, (c t w)]
        in_maps.append(
            {
                "x8": x8,
                "w3": w3_np,
                "b3": b3_np,
                "masks": m_h[h],
            }
        )
    return in_maps


def _gather(results, x, bv):
    # passthrough half assembled on host (exact fp32); device returns the
    # attention read only (bf16), without the v bias (added here: read of
    # (v + bv) = read of v + bv since probs sum to 1)
    out = np.empty((B, S, D + F), dtype=np.float32)
    out[:, :, 0:D] = x
    bv = np.asarray(bv, np.float32).reshape(1, F)
    for c in range(N_CORES):
        b, h = divmod(c, 2)
        # device out is partition-major [p, j, f]
        oc = results[c]["out"].astype(np.float32).transpose(1, 0, 2)
        for j in range(NQT):
            g = 2 * j + h
            out[b, g * 128:(g + 1) * 128, D:] = oc[j] + bv
    return out


def run(inputs, Wq, bq, Wk, bk, Wv, bv, trace=False):
    """Build (cached), run on 8 cores, gather. Returns (output, results)."""
    nc = _get_compiled()
    in_maps = _make_in_maps(inputs, Wq, bq, Wk, bk, Wv, bv)
    x = np.asarray(inputs, dtype=np.float32)
    if trace:
        try:
            res = run_bass_kernel_spmd(nc, in_maps, list(range(N_CORES)), trace=True)
            return _gather(res.results, x, bv), res
        except Exception as e:  # profiling hook unavailable etc.
            print(f"trace run failed ({e!r}); falling back to untraced run")
    res = run_bass_kernel_spmd(nc, in_maps, list(range(N_CORES)))
    return _gather(res.results, x, bv), res


def kernel(inputs, Wq, bq, Wk, bk, Wv, bv):
    out, _ = run(inputs, Wq, bq, Wk, bk, Wv, bv, trace=False)
    return out
